# revision 55
# baseline (speedup 1.0000x reference)
"""GANet SGA kernel for Trainium2, 8 NeuronCores (SPMD).

Model (reference.py): 4-directional semi-global aggregation over a cost
volume x[1,32,48,64,128] with guidance g[1,640,64,128], elementwise max
over directions, BN+ReLU, 3x3x3 conv, BN, residual ReLU.

Wire format (fp16, minimal bytes): x is shipped ONCE, W-sharded and
pre-windowed into AllToAll-input layout (xs [8, 512, 10, 48], halo rows
duplicated, zeros at the edges); all other layouts derive on device:
  - AllToAll #0 reshards x from W-shard to H-shard (10 rows incl 1-row
    halo) straight from the input parameter (collective APs must be
    contiguous on HW); per-row strided DMAs land it in scan layout.
  - Vertical scans (over H) run W-sharded from xs; results scatter into
    the second AllToAll's input by halo-windowed chunk writes.
  - Combine + BN1 (AllReduce stats) + conv (K=128: 4ry x 32ci, M=64:
    2rsh x 32co, 9 (dz,dx) taps) + BN2 (AllReduce) + residual, all
    H-sharded. The conv result stays in SBUF in xh-aligned (row,ch)
    layout so the residual is a direct per-partition add and the BN sel
    masks apply verbatim; output ships fp16 (host transposes).

Scheduling: scan steps are 3 fused custom DVE ops over zero-padded fp16
histories (VectorE is the critical path); PSUM drains run on the Pool
engine; DMAs are spread across the SP/Activation issue queues and the
phase-H loads are emitted before any scan-gated DMA so the round-robin
DMA lanes (in-order each) don't chain them behind the vertical phase.
"""

import sys
import numpy as np
from contextlib import ExitStack

try:
    import concourse.bass  # noqa: F401
except ImportError:
    sys.path.insert(0, "/opt/trn_rl_repo")

C, D, H, W = 32, 48, 64, 128
DP = D + 2                 # padded depth row (zero guard cols 0 and D+1)
HPAD = H + 2               # padded H (zero guard rows 0 and H+1)
NCORES = 8
RPC = H // NCORES          # owned rows per core
RH = RPC + 2               # rows incl halo
CPC = W // NCORES          # cols per core
PV = CPC * C               # vertical-scan partition rows (512)
PH = RH * C                # horizontal-scan partition rows (320)
EPS_BN = 1e-5
EPS_L1 = 1e-12
NTOT = float(D * H * W)    # BN count (B=1)

_prog_cache = {}


# ---------------------------------------------------------------------------
# Custom DVE ops (registered into concourse.dve_ops at import time)
# ---------------------------------------------------------------------------
def _register_custom_ops():
    import concourse.dve_ops as dve_ops
    from concourse.dve_ops import DveOp, OPS, CUSTOM_DVE_SPECS, \
        _SUB_OPCODE_FOR_NAME
    from concourse.dve_spec import (
        Spec, Src0, Src1, C0, C1, MaxNeg, Zero, relu, maxx, lower,
        spec_leaves,
    )
    from concourse.dve_uop import DveOpSpec
    from concourse.dve_table_gen import dve_ver_for
    from operator import add

    if "GANET_DUAL_MAC" in _SUB_OPCODE_FOR_NAME:
        return {n: op for op in OPS for n in [op.name]
                if n.startswith("GANET_")}

    ver = dve_ver_for("TRN2")

    def make(name, spec):
        row = max(_SUB_OPCODE_FOR_NAME.values()) + 1
        tmp = DveOpSpec(name=name, opcode=row, uops=lower(spec, ver=ver),
                        rd1_en=Src1 in spec_leaves(spec))
        op = DveOp(name, spec, subdim=False, uops_sha={ver: tmp.sha(ver)})
        OPS.append(op)
        _SUB_OPCODE_FOR_NAME[name] = row
        CUSTOM_DVE_SPECS[name] = spec
        return op

    def _flat(a, P):
        return np.asarray(a).reshape(P, -1).astype(np.float32)

    def _sc(s, P):
        s = np.asarray(s, np.float32)
        return s.reshape(P, 1) if s.ndim else s

    ops = {}

    def _ref_dual(in0, in1, s0, s1, imm2):
        sh, P = in0.shape, in0.shape[0]
        b = _flat(in0, P) * _sc(s0, P) + _flat(in1, P) * _sc(s1, P)
        return b.astype(np.float32).reshape(sh)

    # out = in0*s0 + in1*s1
    ops["GANET_DUAL_MAC"] = make(
        "GANET_DUAL_MAC",
        Spec(body=Src0 * C0 + Src1 * C1, reference=_ref_dual))

    def _ref_tri(in0, in1, s0, s1, imm2):
        sh, P = in0.shape, in0.shape[0]
        b = (_flat(in0, P) + _flat(in1, P)
             + _sc(s0, P) * _sc(s1, P)).astype(np.float32)
        mx = np.maximum(np.float32(-3.4028234663852886e38),
                        b.max(axis=-1, keepdims=True))
        return b.reshape(sh), mx.astype(np.float32)

    # out = in0 + in1 + s0*s1 ; accum_out = max over free dims
    ops["GANET_TRI_ADD_MAXRED"] = make(
        "GANET_TRI_ADD_MAXRED",
        Spec(body=Src0 + Src1 + C0 * C1, accum=maxx, accum_init=MaxNeg,
             reference=_ref_tri))

    def _ref_affr(in0, in1, s0, s1, imm2):
        sh, P = in0.shape, in0.shape[0]
        b = np.maximum(_flat(in0, P) * _sc(s0, P) + _sc(s1, P), 0)
        return b.astype(np.float32).reshape(sh)

    # out = relu(in0*s0 + s1)   (BN-apply + ReLU)
    ops["GANET_AFF_RELU"] = make(
        "GANET_AFF_RELU",
        Spec(body=relu(Src0 * C0 + C1), reference=_ref_affr))

    def _ref_maxsum(in0, in1, s0, s1, imm2):
        sh, P = in0.shape, in0.shape[0]
        b = np.maximum(_flat(in0, P), _flat(in1, P)).astype(np.float32)
        return b.reshape(sh), b.sum(axis=-1, keepdims=True).astype(np.float32)

    # out = max(in0, in1) ; accum_out = sum over free dims
    ops["GANET_MAX_SUMRED"] = make(
        "GANET_MAX_SUMRED",
        Spec(body=maxx(Src0, Src1), accum=add, accum_init=Zero,
             reference=_ref_maxsum))
    return ops


def _build_program():
    import concourse.bass as bass
    import concourse.tile as tile
    from concourse import mybir
    from concourse.ap import AP

    OPS = _register_custom_ops()
    DUAL = OPS["GANET_DUAL_MAC"]
    TRI = OPS["GANET_TRI_ADD_MAXRED"]
    AFFR = OPS["GANET_AFF_RELU"]
    MAXS = OPS["GANET_MAX_SUMRED"]

    FP = mybir.dt.float32
    FH = mybir.dt.float16
    ALU = mybir.AluOpType
    ACT = mybir.ActivationFunctionType
    AX = mybir.AxisListType

    nc = bass.Bass(num_devices=NCORES)

    # ---------------- DRAM I/O (fp16 wire format) ----------------
    def inp(name, shape, dtype=FH):
        return nc.declare_dram_parameter(name, list(shape), dtype, isOutput=False)

    # x W-shard, pre-windowed into AllToAll-input layout: block j holds
    # rows 8j-1..8j+9 (zeros out of range) for all 512 (col,c) rows.
    xs = inp("xs", (NCORES, PV, RH, D))
    k3v = inp("k3v", (PV, H, 5))
    k4v = inp("k4v", (PV, H, 5))
    k1h = inp("k1h", (PH, W, 5))            # H-shard incl halo rows
    k2h = inp("k2h", (PH, W, 5))
    wstk = inp("wstk", (128, 9, 64))        # conv weights (ry,ci) x (dz,dx) x (rsh,co)
    bn1g = inp("bn1g", (C, 1), FP)
    bn1b = inp("bn1b", (C, 1), FP)
    bn2g = inp("bn2g", (C, 1), FP)
    bn2b = inp("bn2b", (C, 1), FP)
    selg0 = inp("selg0", (128, C), FP)      # BN1 aggregation masks (owned rows)
    selg1 = inp("selg1", (128, C), FP)
    selg2 = inp("selg2", (64, C), FP)
    out_d = nc.declare_dram_parameter("out", [2 * 128, W, D], FH, isOutput=True)

    # Internal DRAM (collective buffers must be contiguous on HW and
    # cannot read IO tensors directly)
    a2ax_in = nc.dram_tensor("a2ax_in", [NCORES, PV, RH, D], FH, kind="Internal")
    a2ax_out = nc.dram_tensor("a2ax_out", [NCORES, PV, RH, D], FH, kind="Internal")
    a2av_in = nc.dram_tensor("a2av_in", [NCORES, PV, RH, D], FH, kind="Internal")
    a2av_out = nc.dram_tensor("a2av_out", [NCORES, PV, RH, D], FH, kind="Internal")
    bn1_in = nc.dram_tensor("bn1_in", [C, 2], FP, kind="Internal")
    bn1_out = nc.dram_tensor("bn1_out", [C, 2], FP, kind="Internal", addr_space="Shared")
    bn2_in = nc.dram_tensor("bn2_in", [C, 2], FP, kind="Internal")
    bn2_out = nc.dram_tensor("bn2_out", [C, 2], FP, kind="Internal", addr_space="Shared")

    RG = [list(range(NCORES))]

    with tile.TileContext(nc) as tc, ExitStack() as top:
        pool_g = top.enter_context(tc.tile_pool(name="pg", bufs=1))   # small persistents
        psum_p = top.enter_context(tc.tile_pool(name="pp", bufs=1, space="PSUM"))

        # AllToAll #0: reshard x (W-shard -> H-shard+halo) from the
        # pre-windowed input parameter. HW collectives can't read IO
        # tensors, so one flat contiguous copy stages it. Emitted first
        # so CC starts immediately.
        nc.sync.dma_start(a2ax_in.ap(), xs.ap())
        nc.gpsimd.collective_compute(
            "AllToAll", ALU.bypass, replica_groups=RG,
            ins=[a2ax_in.ap().opt()], outs=[a2ax_out.ap().opt()],
        )

        def load_reshard(dst_tile, src_dram, rl0, nr, engines):
            """Load [(rl,c), w=(s,col), d] tile rows rl0..rl0+nr from the
            contiguous AllToAll output [s, (col,c), rl, d]. One DMA per
            row (DMA AP balancing caps at 3 dims), issued round-robin on
            `engines`' DMA queues."""
            src5 = src_dram.ap().rearrange("s (col c) rl d -> rl c s col d",
                                           c=C)
            for rl in range(nr):
                dst = dst_tile[C * rl:C * (rl + 1)] \
                    .rearrange("c (s col) d -> c s col d", col=CPC)
                engines[rl % len(engines)].dma_start(dst, src5[rl0 + rl])

        # ---------- helpers ----------
        def normalize_k(pool, kt, P, L, tag, kf_pool=None):
            """L1-normalize fp16 kt [P, L, 5] along the 5 axis into a new
            fp32 tile (scan scalar operands must be fp32)."""
            kf_pool = kf_pool or pool
            den = pool.tile([P, L], FP, name=f"den_{tag}", tag="den", bufs=3)
            nc.vector.tensor_reduce(
                out=den[:], in_=kt[:], axis=AX.X, op=ALU.add,
                apply_absolute_value=True,
            )
            nc.vector.tensor_scalar_max(den[:], den[:], EPS_L1)
            rec = pool.tile([P, L], FP, name=f"rec_{tag}", tag="rec", bufs=3)
            nc.vector.reciprocal(rec[:], den[:])
            kf = kf_pool.tile([P, L, 5], FP, name=f"kf_{tag}", tag=f"kf_{tag}")
            for j in range(5):
                nc.vector.tensor_mul(kf[:, :, j], kt[:, :, j], rec[:])
            return kf

        scan_counter = [0]

        def emit_scan(pool, x_g, kn_g, hist_g, L, reverse, P):
            """Sequential SGA scan along axis 'L' of x_g [P, L, D].

            hist_g [P, L, DP] fp16 holds the scan history with zero guard
            columns 0 and D+1 (pre-zeroed by caller). Per step: 3 fused
            custom DVE ops, all on VectorE.
            """
            cid = scan_counter[0]
            scan_counter[0] += 1
            mxA = pool.tile([P, 1], FP, name=f"mxA{cid}", tag=f"mxA{cid}")
            mxB = pool.tile([P, 1], FP, name=f"mxB{cid}", tag=f"mxB{cid}")
            at = pool.tile([P, D], FP, name=f"a{cid}", tag=f"a{cid}")
            bt = pool.tile([P, D], FP, name=f"b{cid}", tag=f"b{cid}")

            pos0 = L - 1 if reverse else 0
            nc.vector.tensor_copy(hist_g[:, pos0, 1:D + 1], x_g[:, pos0, :])
            nc.vector.tensor_reduce(out=mxA[:], in_=hist_g[:, pos0, 1:D + 1],
                                    axis=AX.X, op=ALU.max)

            mx_prev, mx_next = mxA, mxB
            for s in range(1, L):
                t = L - 1 - s if reverse else s
                tp = t + 1 if reverse else t - 1
                hp = hist_g[:, tp, :]          # padded prev row [P, DP]
                k0 = kn_g[:, t, 0:1]
                k1 = kn_g[:, t, 1:2]
                k2 = kn_g[:, t, 2:3]
                k3 = kn_g[:, t, 3:4]
                k4 = kn_g[:, t, 4:5]
                # a = x_t*w0 + prev*w2
                nc.vector._custom_dve(
                    DUAL, out=at[:], in0=x_g[:, t, :], in1=hp[:, 1:D + 1],
                    s0=k0, s1=k2)
                # b = prev[d-1]*w1 + prev[d+1]*w3 (guards make edges exact)
                nc.vector._custom_dve(
                    DUAL, out=bt[:], in0=hp[:, 0:D], in1=hp[:, 2:DP],
                    s0=k1, s1=k3)
                # cur = a + b + mx*w4 ; mx_next = max_d(cur)
                nc.vector._custom_dve(
                    TRI, out=hist_g[:, t, 1:D + 1], in0=at[:], in1=bt[:],
                    s0=mx_prev[:], s1=k4, accum_out=mx_next[:])
                mx_prev, mx_next = mx_next, mx_prev

        def zero_guards(hist_g, P, L):
            nc.vector.memset(hist_g[:, :, 0:1], 0.0)
            nc.vector.memset(hist_g[:, :, D + 1:DP], 0.0)

        # =======================================================
        # Phase V: vertical scans (W-sharded), feed AllToAll #1.
        # All loads are emitted up front: the SP queue is in-order, so
        # the phase-H loads must not sit behind the scan-gated result
        # writes; phase-H tiles load on the Scalar DMA queue instead.
        # =======================================================
        zrow = pool_g.tile([128, 1, D], FH, name="zrow", tag="zrow")
        nc.vector.memset(zrow[:], 0.0)
        pe = top.enter_context(tc.tile_pool(name="pe", bufs=1))
        HG = [(0, 128), (128, 128), (256, 64)]  # (row-part offset, partitions)
        xh_tiles, k12_tiles, kh_f = [], [], []
        with tc.tile_pool(name="pv", bufs=1) as pv:
            xv_tiles, k3_tiles, k4_tiles = [], [], []
            for g in range(4):
                # x rows for the vertical scan, gathered from the
                # windowed layout (row h lives in block h//8 at 1 + h%8)
                xv_g = pv.tile([128, H, D], FH, name=f"xv{g}", tag=f"xv{g}")
                src = AP(xs, (128 * g * RH + 1) * D,
                         [[RH * D, 128], [PV * RH * D, NCORES], [D, RPC],
                          [1, D]])
                nc.sync.dma_start(xv_g[:], src)
                xv_tiles.append(xv_g)
                k3_g = pv.tile([128, H, 5], FH, name=f"k3{g}", tag=f"k3{g}")
                nc.sync.dma_start(k3_g[:], k3v[128 * g:128 * (g + 1), :, :])
                k4_g = pv.tile([128, H, 5], FH, name=f"k4{g}", tag=f"k4{g}")
                nc.sync.dma_start(k4_g[:], k4v[128 * g:128 * (g + 1), :, :])
                k3_tiles.append(k3_g)
                k4_tiles.append(k4_g)
            # phase-H loads, emitted BEFORE any scan-gated DMA: the DMA
            # lanes (DMAHW0..7) are assigned round-robin in emission
            # order and each lane is in-order, so emitting these after
            # the chunk writes would chain them behind the V scans.
            for g, (p0, P) in enumerate(HG):
                xh_g = pe.tile([P, W, D], FH, name=f"xh{g}", tag=f"xh{g}")
                load_reshard(xh_g, a2ax_out, p0 // C, P // C,
                             engines=[nc.scalar])
                xh_tiles.append(xh_g)
                k1_g = pv.tile([P, W, 5], FH, name=f"k1{g}", tag=f"k1{g}")
                nc.scalar.dma_start(k1_g[:], k1h[p0:p0 + P, :, :])
                k2_g = pv.tile([P, W, 5], FH, name=f"k2{g}", tag=f"k2{g}")
                nc.scalar.dma_start(k2_g[:], k2h[p0:p0 + P, :, :])
                k12_tiles.append((k1_g, k2_g))
            for g in range(4):
                xv_g, k3_g, k4_g = xv_tiles[g], k3_tiles[g], k4_tiles[g]
                k3f = normalize_k(pv, k3_g, 128, H, f"v3{g}")
                k4f = normalize_k(pv, k4_g, 128, H, f"v4{g}")
                a3_g = pv.tile([128, H, DP], FH, name=f"a3{g}", tag=f"a3{g}")
                h4_g = pv.tile([128, H, DP], FH, name=f"h4{g}", tag=f"h4{g}")
                zero_guards(a3_g, 128, H)
                zero_guards(h4_g, 128, H)
                emit_scan(pv, xv_g, k3f, a3_g, H, reverse=False, P=128)
                emit_scan(pv, xv_g, k4f, h4_g, H, reverse=True, P=128)
                nc.vector.tensor_max(a3_g[:], a3_g[:], h4_g[:])
                # scatter result chunks into the AllToAll input (1-row
                # halo each side; zero out-of-range rows)
                for j in range(NCORES):
                    h0 = RPC * j - 1
                    dst = a2av_in[j, 128 * g:128 * (g + 1), :, :]
                    if h0 < 0:
                        nc.sync.dma_start(dst[:, 1:RH, :],
                                          a3_g[:, 0:RH - 1, 1:D + 1])
                        nc.sync.dma_start(dst[:, 0:1, :], zrow[:])
                    elif h0 + RH > H:
                        nc.sync.dma_start(dst[:, 0:RH - 1, :],
                                          a3_g[:, h0:H, 1:D + 1])
                        nc.sync.dma_start(dst[:, RH - 1:RH, :], zrow[:])
                    else:
                        nc.sync.dma_start(dst, a3_g[:, h0:h0 + RH, 1:D + 1])
            # normalize the horizontal k's here (after the V scans in DVE
            # queue order, just before the H scans need them); raw tiles
            # die with this pool, normalized fp32 tiles live in ph.
            for g, (p0, P) in enumerate(HG):
                k1_g, k2_g = k12_tiles[g]
                kh_f.append((normalize_k(pv, k1_g, P, W, f"h1{g}", kf_pool=pe),
                             normalize_k(pv, k2_g, P, W, f"h2{g}", kf_pool=pe)))

        nc.gpsimd.collective_compute(
            "AllToAll", ALU.bypass, replica_groups=RG,
            ins=[a2av_in.ap().opt()], outs=[a2av_out.ap().opt()],
        )

        # =======================================================
        # Phase H: horizontal scans (H-sharded, 10 rows w/ halo)
        # Overlaps AllToAll #1 (only depends on AllToAll #0).
        # =======================================================
        with ExitStack() as hs:
            ph = hs.enter_context(tc.tile_pool(name="ph", bufs=1))
            a1_tiles = []
            with tc.tile_pool(name="phx", bufs=1) as phx:
                for g, (p0, P) in enumerate(HG):
                    xh_g = xh_tiles[g]
                    k1f, k2f = kh_f[g]
                    a1_g = ph.tile([P, W, DP], FH, name=f"a1{g}", tag=f"a1{g}")
                    h2_g = phx.tile([P, W, DP], FH, name=f"h2{g}", tag=f"h2{g}")
                    zero_guards(a1_g, P, W)
                    zero_guards(h2_g, P, W)
                    emit_scan(phx, xh_g, k1f, a1_g, W, reverse=False, P=P)
                    emit_scan(phx, xh_g, k2f, h2_g, W, reverse=True, P=P)
                    nc.vector.tensor_max(a1_g[:], a1_g[:], h2_g[:])
                    a1_tiles.append(a1_g)

            # ===================================================
            # Combine + BN1 stats
            # ===================================================
            pcv = hs.enter_context(tc.tile_pool(name="pcv", bufs=1))
            bnp = psum_p.tile([C, 2], FP, name="bnp", tag="bnp")
            sel_tiles = []
            for nm, ap_, P in (("s0", selg0, 128), ("s1", selg1, 128), ("s2", selg2, 64)):
                selt = pool_g.tile([P, C], FP, name=f"sel{nm}", tag=f"sel{nm}")
                nc.sync.dma_start(selt[:], ap_[:])
                sel_tiles.append(selt)

            for g, (p0, P) in enumerate(HG):
                a1_g = a1_tiles[g]
                a34_g = ph.tile([P, W, D], FH, name=f"a34_{g}", tag="a34", bufs=2)
                load_reshard(a34_g, a2av_out, p0 // C, P // C,
                             engines=[nc.scalar, nc.sync])
                s12_g = pool_g.tile([P, 2], FP, name=f"bns{g}", tag=f"bns{g}")
                # ymax = max(a12, a34) in place, fused sum accumulate
                nc.vector._custom_dve(
                    MAXS, out=a1_g[:, :, 1:D + 1], in0=a1_g[:, :, 1:D + 1],
                    in1=a34_g[:], s0=0.0, s1=0.0, accum_out=s12_g[:, 0:1])
                # sum of squares (ScalarE; scratch output reuses a34_g)
                nc.scalar.activation(a34_g[:], a1_g[:, :, 1:D + 1], ACT.Square,
                                     accum_out=s12_g[:, 1:2])
                nc.tensor.matmul(bnp[:], lhsT=sel_tiles[g][:], rhs=s12_g[:],
                                 start=(g == 0), stop=(g == 2))

            bnst = pool_g.tile([C, 2], FP, name="bnst", tag="bnst")
            nc.vector.tensor_copy(bnst[:], bnp[:])
            nc.sync.dma_start(bn1_in[:], bnst[:])
            nc.gpsimd.collective_compute(
                "AllReduce", ALU.add, replica_groups=RG,
                ins=[bn1_in.ap().opt()], outs=[bn1_out.ap().opt()],
            )

            def bn_scale_bias(bn_out_dram, gam, bet, tag):
                """([C,1] scale, [C,1] bias) from AllReduce'd (sum, sumsq)."""
                st = pool_g.tile([C, 2], FP, name=f"bnr{tag}", tag=f"bnr{tag}")
                nc.sync.dma_start(st[:], bn_out_dram[:])
                gt = pool_g.tile([C, 1], FP, name=f"bng{tag}", tag=f"bng{tag}")
                nc.sync.dma_start(gt[:], gam[:])
                bt = pool_g.tile([C, 1], FP, name=f"bnb{tag}", tag=f"bnb{tag}")
                nc.sync.dma_start(bt[:], bet[:])
                mean = pool_g.tile([C, 1], FP, name=f"mean{tag}", tag=f"mean{tag}")
                nc.vector.tensor_scalar_mul(mean[:], st[:, 0:1], 1.0 / NTOT)
                var = pool_g.tile([C, 1], FP, name=f"var{tag}", tag=f"var{tag}")
                nc.vector.tensor_scalar_mul(var[:], st[:, 1:2], 1.0 / NTOT)
                msq = pool_g.tile([C, 1], FP, name=f"msq{tag}", tag=f"msq{tag}")
                nc.vector.tensor_mul(msq[:], mean[:], mean[:])
                nc.vector.tensor_tensor(out=var[:], in0=var[:], in1=msq[:],
                                        op=ALU.subtract)
                nc.vector.tensor_scalar_add(var[:], var[:], EPS_BN)
                sd = pool_g.tile([C, 1], FP, name=f"sd{tag}", tag=f"sd{tag}")
                nc.scalar.activation(sd[:], var[:], ACT.Sqrt)
                rs = pool_g.tile([C, 1], FP, name=f"rs{tag}", tag=f"rs{tag}")
                nc.vector.reciprocal(rs[:], sd[:])
                sc = pool_g.tile([C, 1], FP, name=f"sc{tag}", tag=f"sc{tag}")
                nc.vector.tensor_mul(sc[:], rs[:], gt[:])
                bi = pool_g.tile([C, 1], FP, name=f"bi{tag}", tag=f"bi{tag}")
                nc.vector.tensor_mul(bi[:], mean[:], sc[:])
                nc.vector.tensor_tensor(out=bi[:], in0=bt[:], in1=bi[:],
                                        op=ALU.subtract)
                return sc, bi

            sc1, bi1 = bn_scale_bias(bn1_out, bn1g, bn1b, "1")
            sc1b = pool_g.tile([128, 1], FP, name="sc1b", tag="sc1b")
            bi1b = pool_g.tile([128, 1], FP, name="bi1b", tag="bi1b")
            for b in range(4):
                nc.sync.dma_start(sc1b[C * b:C * (b + 1), :], sc1[:])
                nc.sync.dma_start(bi1b[C * b:C * (b + 1), :], bi1[:])
            # y = relu(scale*ymax + bias), in place (halo rows included)
            for g, (p0, P) in enumerate(HG):
                nc.vector._custom_dve(
                    AFFR, out=a1_tiles[g][:, :, 1:D + 1],
                    in0=a1_tiles[g][:, :, 1:D + 1],
                    s0=sc1b[0:P, :], s1=bi1b[0:P, :])

            # ===================================================
            # Conv 3x3x3, row-folded: K=128 (4ry x 32ci), M=64
            # (2rsh x 32co), 9 (dz,dx) taps, D in 3 chunks of 16.
            # Result lands in SBUF tiles conv_sbuf[g] [(4row,32ch),W,D].
            # ===================================================
            wst = pool_g.tile([128, 9, 64], FH, name="wst", tag="wst")
            nc.sync.dma_start(wst[:], wstk[:])
            # conv result tiles, indexed like the xh halo window (block =
            # xh local row % 4, tile = row // 4; out row R <-> xh row R+1)
            # so the residual add and the BN1 sel masks line up directly.
            conv_sbuf = [
                ph.tile([P, W, D], FH, name=f"cs{g}", tag=f"cs{g}")
                for g, (_, P) in enumerate(HG)
            ]
            nc.vector.memset(conv_sbuf[0][0:C], 0.0)    # unused halo blocks
            nc.vector.memset(conv_sbuf[2][C:2 * C], 0.0)

            DC = 16
            DX = DC + 2                      # depth window incl dz shifts
            WP = W + 2
            taps = [(a, b) for a in range(3) for b in range(3)]
            for ch in range(3):
                d0 = DC * ch
                # y5[(ry,ci), rb, wp, dpx] = ypad[ci, d0+dpx-1, rb*2+ry, wp-1]
                # (dpx alloc 19, 18 used, to block wp/dpx AP coalescing; a1
                # zero-guard cols provide the depth edge padding)
                y5 = pcv.tile([128, 4, WP, DX + 1], FH, name=f"y5_{ch}",
                              tag="y5", bufs=2)
                nc.vector.memset(y5[:, :, 0:1, :], 0.0)
                nc.vector.memset(y5[:, :, WP - 1:WP, :], 0.0)
                for rb in range(4):
                    rbase = 1 + 2 * rb       # local a1 row of first output
                    for ry in range(4):
                        rr = rbase + ry - 1  # source local row (0..9)
                        g = rr // 4
                        rl = rr - 4 * g
                        src = a1_tiles[g][C * rl:C * (rl + 1), :, d0:d0 + DX]
                        dst = y5[32 * ry:32 * (ry + 1), rb, 1:W + 1, 0:DX]
                        eng = nc.sync if (rb % 2 == 0) else nc.scalar
                        eng.dma_start(dst, src)
                for rb in range(4):
                    for dps in range(4):     # 4 depths per matmul group
                        pt = psum_p.tile([64, W, 4], FP, name="cps", tag="cps",
                                         bufs=6)
                        for ti, (dz, dx) in enumerate(taps):
                            rhs = y5[:, rb, dx:dx + W,
                                     4 * dps + dz:4 * dps + dz + 4]
                            nc.tensor.matmul(pt[:], lhsT=wst[:, ti, :],
                                             rhs=rhs, start=(ti == 0),
                                             stop=(ti == 8))
                        dlo = d0 + 4 * dps
                        for rsh in range(2):
                            rr = 2 * rb + rsh + 1      # xh-window row (1..8)
                            go, prow = rr // 4, (rr % 4) * C
                            # PSUM drain (GPSIMD cannot access PSUM on
                            # HW, so this stays on ScalarE)
                            nc.scalar.copy(
                                conv_sbuf[go][prow:prow + C, :, dlo:dlo + 4],
                                pt[32 * rsh:32 * (rsh + 1), :, :])

            # ===================================================
            # BN2 stats + final residual (layout [(4row,32ch),W,D])
            # ===================================================
            bnp2 = psum_p.tile([C, 2], FP, name="bnp2", tag="bnp2")
            for g, (p0, P) in enumerate(HG):
                sAB = pool_g.tile([P, 2], FP, name=f"sAB{g}", tag=f"sAB{g}")
                nc.vector.tensor_reduce(out=sAB[:, 0:1], in_=conv_sbuf[g][:],
                                        axis=AX.XY, op=ALU.add)
                sqs = ph.tile([P, W, D], FH, name=f"sq{g}", tag="a34",
                              bufs=2)
                nc.scalar.activation(sqs[:], conv_sbuf[g][:], ACT.Square,
                                     accum_out=sAB[:, 1:2])
                nc.tensor.matmul(bnp2[:], lhsT=sel_tiles[g][:], rhs=sAB[:],
                                 start=(g == 0), stop=(g == 2))
            bnst2 = pool_g.tile([C, 2], FP, name="bnst2", tag="bnst2")
            nc.vector.tensor_copy(bnst2[:], bnp2[:])
            nc.sync.dma_start(bn2_in[:], bnst2[:])
            nc.gpsimd.collective_compute(
                "AllReduce", ALU.add, replica_groups=RG,
                ins=[bn2_in.ap().opt()], outs=[bn2_out.ap().opt()],
            )
            sc2, bi2 = bn_scale_bias(bn2_out, bn2g, bn2b, "2")
            sc2b = pool_g.tile([128, 1], FP, name="sc2b", tag="sc2b")
            bi2b = pool_g.tile([128, 1], FP, name="bi2b", tag="bi2b")
            for b in range(4):
                nc.sync.dma_start(sc2b[C * b:C * (b + 1), :], sc2[:])
                nc.sync.dma_start(bi2b[C * b:C * (b + 1), :], bi2[:])

            # out = relu(sc2*conv + rem + bi2); conv tiles are xh-aligned,
            # so the residual is a direct per-partition add. Partition
            # quadrant rule: ops may start only at 0/32/64/96 with <=
            # 128/32/64/32 partitions, so cs0's valid span 32..128 splits.
            def stt(cs, xh_t, lo, hi):
                nc.vector.scalar_tensor_tensor(
                    out=cs[lo:hi], in0=cs[lo:hi], scalar=sc2b[lo:hi],
                    in1=xh_t[lo:hi], op0=ALU.mult, op1=ALU.add,
                )
            stt(conv_sbuf[0], xh_tiles[0], C, 2 * C)
            stt(conv_sbuf[0], xh_tiles[0], 2 * C, 4 * C)
            stt(conv_sbuf[1], xh_tiles[1], 0, 4 * C)
            stt(conv_sbuf[2], xh_tiles[2], 0, C)
            for g, (p0, P) in enumerate(HG):
                nc.scalar.activation(conv_sbuf[g][:], conv_sbuf[g][:],
                                     ACT.Relu, bias=bi2b[0:P], scale=1.0)
            # ship owned rows only (xh rows 1..8 = out rows 0..7),
            # spread across the three DMA-capable engine queues
            nc.sync.dma_start(out_d[0:96], conv_sbuf[0][C:4 * C])
            nc.scalar.dma_start(out_d[96:224], conv_sbuf[1][:])
            nc.gpsimd.dma_start(out_d[224:256], conv_sbuf[2][0:C])

    return nc


def _split_sync_waits(nc, maxw=1):
    """Walrus codegen rejects >1 sem wait per instruction (TPB_CTRL limit);
    hoist extras onto fresh NoOps on the same engine just before the owner."""
    from concourse import mybir
    n_split = 0
    for bb in nc.main_func.blocks:
        i = 0
        insts = bb.instructions
        while i < len(insts):
            ins = insts[i]
            si = getattr(ins, "sync_info", None)
            if si is not None and len(si.on_wait) > maxw:
                waits = list(si.on_wait)
                si.on_wait = waits[:maxw]
                extra = waits[maxw:]
                new_nops = []
                for j in range(0, len(extra), maxw):
                    nop = mybir.InstNoOp(
                        name=nc.get_next_instruction_name(),
                        sync_info=mybir.SyncInfo(
                            on_wait=extra[j:j + maxw], on_update=[]),
                        bass_nofuse=True,
                        engine=ins.engine,
                    )
                    nc.register_instruction(nop)
                    new_nops.append(nop)
                for k, nop in enumerate(new_nops):
                    insts.insert(i + k, nop)
                i += len(new_nops)
                n_split += 1
            i += 1
    return n_split


def _get_program():
    if "nc" not in _prog_cache:
        from concourse import mybir
        nc = _build_program()
        _split_sync_waits(nc)
        # Raw Bass doesn't encode InstISA subclasses (custom DVE etc.);
        # walrus then sees empty .instr -> "ISA wrong length".
        mybir.codegen_inst_isa_subclasses(nc)
        _prog_cache["nc"] = nc
    return _prog_cache["nc"]


def _prep_inputs(x, g, conv_w, bn1_gamma, bn1_beta, bn2_gamma, bn2_beta):
    """Host-side sharding + layout transposes + fp16 cast (no arithmetic)."""
    x = np.asarray(x, np.float32)[0]            # [C,D,H,W]
    g = np.asarray(g, np.float32)[0]            # [640,H,W]
    conv_w = np.asarray(conv_w, np.float32)
    ks = g.reshape(4, C, 5, H, W)               # k1..k4

    # conv weight stack, row-folded:
    # wstk[ry*32+ci, dz*3+dx, rsh*32+co] = conv_w[co,ci,dz,ry-rsh,dx]
    wstk = np.zeros((128, 9, 64), np.float32)
    for ry in range(4):
        for rsh in range(2):
            dy = ry - rsh
            if 0 <= dy <= 2:
                # [ci, (dz,dx), co]
                wblk = conv_w[:, :, :, dy, :].transpose(1, 2, 3, 0) \
                    .reshape(C, 9, C)
                wstk[32 * ry:32 * (ry + 1), :, 32 * rsh:32 * (rsh + 1)] = wblk
    wstk = wstk.astype(np.float16)

    # BN1 masks per h-group (owned local rows are 1..8 of 0..9)
    def sel(nrows, r_base):
        m = np.zeros((nrows * C, C), np.float32)
        for rl in range(nrows):
            r = r_base + rl
            if 1 <= r <= 8:
                for c in range(C):
                    m[rl * C + c, c] = 1.0
        return m
    selg0, selg1, selg2 = sel(4, 0), sel(4, 4), sel(2, 8)

    maps = []
    for i in range(NCORES):
        r_lo, r_hi = 8 * i - 1, 8 * i + 9
        k1p = np.zeros((RH, C, W, 5), np.float16)
        k2p = np.zeros((RH, C, W, 5), np.float16)
        lo, hi = max(r_lo, 0), min(r_hi, H)
        k1p[lo - r_lo:hi - r_lo] = ks[0][:, :, lo:hi, :].transpose(2, 0, 3, 1)
        k2p[lo - r_lo:hi - r_lo] = ks[1][:, :, lo:hi, :].transpose(2, 0, 3, 1)
        cs = slice(CPC * i, CPC * (i + 1))
        xpad = np.zeros((PV, H + 2, D), np.float16)
        xpad[:, 1:H + 1, :] = x[:, :, :, cs].transpose(3, 0, 2, 1) \
            .reshape(PV, H, D)
        widx = RPC * np.arange(NCORES)[:, None] + np.arange(RH)[None, :]
        xsw = np.ascontiguousarray(xpad[:, widx, :].transpose(1, 0, 2, 3))
        k3s = ks[2][:, :, :, cs].transpose(3, 0, 2, 1).reshape(PV, H, 5)
        k4s = ks[3][:, :, :, cs].transpose(3, 0, 2, 1).reshape(PV, H, 5)
        maps.append({
            "xs": xsw,
            "k1h": np.ascontiguousarray(k1p.reshape(PH, W, 5)),
            "k2h": np.ascontiguousarray(k2p.reshape(PH, W, 5)),
            "k3v": np.ascontiguousarray(k3s.astype(np.float16)),
            "k4v": np.ascontiguousarray(k4s.astype(np.float16)),
            "wstk": wstk,
            "bn1g": np.asarray(bn1_gamma, np.float32).reshape(C, 1),
            "bn1b": np.asarray(bn1_beta, np.float32).reshape(C, 1),
            "bn2g": np.asarray(bn2_gamma, np.float32).reshape(C, 1),
            "bn2b": np.asarray(bn2_beta, np.float32).reshape(C, 1),
            "selg0": selg0, "selg1": selg1, "selg2": selg2,
        })
    return maps


def _assemble(results):
    """Per-core out [256, W, D] fp16 ((row,ch) layout) -> full [1,C,D,H,W]."""
    full = np.zeros((1, C, D, H, W), np.float32)
    for i, r in enumerate(results):
        o = np.asarray(r["out"]).astype(np.float32) \
             .reshape(RPC, C, W, D).transpose(1, 3, 0, 2)
        full[0, :, :, 8 * i:8 * (i + 1), :] = o
    return full


def _run_sim(nc, maps):
    """Fallback: numerically-exact multi-core simulator (no hardware)."""
    from concourse import bass_interp
    sim = bass_interp.MultiCoreSim(nc, NCORES)
    for i in range(NCORES):
        for name, arr in maps[i].items():
            sim.cores[i].tensor(name)[:] = arr
    sim.simulate(check_with_hw=False)
    return [{"out": sim.cores[i].mem_tensor("out")} for i in range(NCORES)]


def kernel(x, g, conv_w, bn1_gamma, bn1_beta, bn2_gamma, bn2_beta):
    from concourse.bass_utils import run_bass_kernel_spmd
    nc = _get_program()
    maps = _prep_inputs(x, g, conv_w, bn1_gamma, bn1_beta, bn2_gamma, bn2_beta)
    try:
        res = run_bass_kernel_spmd(nc, maps, list(range(NCORES)))
        results = res.results
    except Exception as e:  # axon/PJRT unavailable -> simulate
        print(f"kernel: hardware path failed ({type(e).__name__}: {e}); "
              f"falling back to MultiCoreSim", file=sys.stderr)
        results = _run_sim(nc, maps)
    return _assemble(results)


if __name__ == "__main__":
    nc = _build_program()
    print("program built OK; instructions:",
          sum(len(bb.instructions) for bb in nc.main_func.blocks))


# revision 56
# speedup vs baseline: 1.0356x; 1.0356x over previous
"""GANet SGA kernel for Trainium2, 8 NeuronCores (SPMD).

Model (reference.py): 4-directional semi-global aggregation over a cost
volume x[1,32,48,64,128] with guidance g[1,640,64,128], elementwise max
over directions, BN+ReLU, 3x3x3 conv, BN, residual ReLU.

Wire format (fp16, minimal bytes): x is shipped ONCE, W-sharded and
pre-windowed into AllToAll-input layout (xs [8, 512, 10, 48], halo rows
duplicated, zeros at the edges); all other layouts derive on device:
  - AllToAll #0 reshards x from W-shard to H-shard (10 rows incl 1-row
    halo) straight from the input parameter (collective APs must be
    contiguous on HW); per-row strided DMAs land it in scan layout.
  - Vertical scans (over H) run W-sharded from xs; results scatter into
    the second AllToAll's input by halo-windowed chunk writes.
  - Combine + BN1 (AllReduce stats) + conv (K=128: 4ry x 32ci, M=64:
    2rsh x 32co, 9 (dz,dx) taps) + BN2 (AllReduce) + residual, all
    H-sharded. The conv result stays in SBUF in xh-aligned (row,ch)
    layout so the residual is a direct per-partition add and the BN sel
    masks apply verbatim; output ships fp16 (host transposes).

Scheduling: scan steps are 3 fused custom DVE ops over zero-padded fp16
histories (VectorE is the critical path); PSUM drains run on the Pool
engine; DMAs are spread across the SP/Activation issue queues and the
phase-H loads are emitted before any scan-gated DMA so the round-robin
DMA lanes (in-order each) don't chain them behind the vertical phase.
"""

import sys
import numpy as np
from contextlib import ExitStack

try:
    import concourse.bass  # noqa: F401
except ImportError:
    sys.path.insert(0, "/opt/trn_rl_repo")

C, D, H, W = 32, 48, 64, 128
DP = D + 2                 # padded depth row (zero guard cols 0 and D+1)
HPAD = H + 2               # padded H (zero guard rows 0 and H+1)
NCORES = 8
RPC = H // NCORES          # owned rows per core
RH = RPC + 2               # rows incl halo
CPC = W // NCORES          # cols per core
PV = CPC * C               # vertical-scan partition rows (512)
PH = RH * C                # horizontal-scan partition rows (320)
EPS_BN = 1e-5
EPS_L1 = 1e-12
NTOT = float(D * H * W)    # BN count (B=1)

_prog_cache = {}


# ---------------------------------------------------------------------------
# Custom DVE ops (registered into concourse.dve_ops at import time)
# ---------------------------------------------------------------------------
def _register_custom_ops():
    import concourse.dve_ops as dve_ops
    from concourse.dve_ops import DveOp, OPS, CUSTOM_DVE_SPECS, \
        _SUB_OPCODE_FOR_NAME
    from concourse.dve_spec import (
        Spec, Src0, Src1, C0, C1, MaxNeg, Zero, relu, maxx, lower,
        spec_leaves,
    )
    from concourse.dve_uop import DveOpSpec
    from concourse.dve_table_gen import dve_ver_for
    from operator import add

    if "GANET_DUAL_MAC" in _SUB_OPCODE_FOR_NAME:
        return {n: op for op in OPS for n in [op.name]
                if n.startswith("GANET_")}

    ver = dve_ver_for("TRN2")

    def make(name, spec):
        row = max(_SUB_OPCODE_FOR_NAME.values()) + 1
        tmp = DveOpSpec(name=name, opcode=row, uops=lower(spec, ver=ver),
                        rd1_en=Src1 in spec_leaves(spec))
        op = DveOp(name, spec, subdim=False, uops_sha={ver: tmp.sha(ver)})
        OPS.append(op)
        _SUB_OPCODE_FOR_NAME[name] = row
        CUSTOM_DVE_SPECS[name] = spec
        return op

    def _flat(a, P):
        return np.asarray(a).reshape(P, -1).astype(np.float32)

    def _sc(s, P):
        s = np.asarray(s, np.float32)
        return s.reshape(P, 1) if s.ndim else s

    ops = {}

    def _ref_dual(in0, in1, s0, s1, imm2):
        sh, P = in0.shape, in0.shape[0]
        b = _flat(in0, P) * _sc(s0, P) + _flat(in1, P) * _sc(s1, P)
        return b.astype(np.float32).reshape(sh)

    # out = in0*s0 + in1*s1
    ops["GANET_DUAL_MAC"] = make(
        "GANET_DUAL_MAC",
        Spec(body=Src0 * C0 + Src1 * C1, reference=_ref_dual))

    def _ref_tri(in0, in1, s0, s1, imm2):
        sh, P = in0.shape, in0.shape[0]
        b = (_flat(in0, P) + _flat(in1, P)
             + _sc(s0, P) * _sc(s1, P)).astype(np.float32)
        mx = np.maximum(np.float32(-3.4028234663852886e38),
                        b.max(axis=-1, keepdims=True))
        return b.reshape(sh), mx.astype(np.float32)

    # out = in0 + in1 + s0*s1 ; accum_out = max over free dims
    ops["GANET_TRI_ADD_MAXRED"] = make(
        "GANET_TRI_ADD_MAXRED",
        Spec(body=Src0 + Src1 + C0 * C1, accum=maxx, accum_init=MaxNeg,
             reference=_ref_tri))

    def _ref_affr(in0, in1, s0, s1, imm2):
        sh, P = in0.shape, in0.shape[0]
        b = np.maximum(_flat(in0, P) * _sc(s0, P) + _sc(s1, P), 0)
        return b.astype(np.float32).reshape(sh)

    # out = relu(in0*s0 + s1)   (BN-apply + ReLU)
    ops["GANET_AFF_RELU"] = make(
        "GANET_AFF_RELU",
        Spec(body=relu(Src0 * C0 + C1), reference=_ref_affr))

    def _ref_maxsum(in0, in1, s0, s1, imm2):
        sh, P = in0.shape, in0.shape[0]
        b = np.maximum(_flat(in0, P), _flat(in1, P)).astype(np.float32)
        return b.reshape(sh), b.sum(axis=-1, keepdims=True).astype(np.float32)

    # out = max(in0, in1) ; accum_out = sum over free dims
    ops["GANET_MAX_SUMRED"] = make(
        "GANET_MAX_SUMRED",
        Spec(body=maxx(Src0, Src1), accum=add, accum_init=Zero,
             reference=_ref_maxsum))
    return ops


def _build_program():
    import concourse.bass as bass
    import concourse.tile as tile
    from concourse import mybir
    from concourse.ap import AP

    OPS = _register_custom_ops()
    DUAL = OPS["GANET_DUAL_MAC"]
    TRI = OPS["GANET_TRI_ADD_MAXRED"]
    AFFR = OPS["GANET_AFF_RELU"]
    MAXS = OPS["GANET_MAX_SUMRED"]

    FP = mybir.dt.float32
    FH = mybir.dt.float16
    ALU = mybir.AluOpType
    ACT = mybir.ActivationFunctionType
    AX = mybir.AxisListType

    nc = bass.Bass(num_devices=NCORES)

    # ---------------- DRAM I/O (fp16 wire format) ----------------
    def inp(name, shape, dtype=FH):
        return nc.declare_dram_parameter(name, list(shape), dtype, isOutput=False)

    # x W-shard, pre-windowed into AllToAll-input layout: block j holds
    # rows 8j-1..8j+9 (zeros out of range) for all 512 (col,c) rows.
    xs = inp("xs", (NCORES, PV, RH, D))
    k3v = inp("k3v", (PV, H, 5))
    k4v = inp("k4v", (PV, H, 5))
    k1h = inp("k1h", (PH, W, 5))            # H-shard incl halo rows
    k2h = inp("k2h", (PH, W, 5))
    wstk = inp("wstk", (128, 9, 64))        # conv weights (ry,ci) x (dz,dx) x (rsh,co)
    bn1g = inp("bn1g", (C, 1), FP)
    bn1b = inp("bn1b", (C, 1), FP)
    bn2g = inp("bn2g", (C, 1), FP)
    bn2b = inp("bn2b", (C, 1), FP)
    selg0 = inp("selg0", (128, C), FP)      # BN1 aggregation masks (owned rows)
    selg1 = inp("selg1", (128, C), FP)
    selg2 = inp("selg2", (64, C), FP)
    out_d = nc.declare_dram_parameter("out", [2 * 128, W, D], FH, isOutput=True)

    # Internal DRAM (collective buffers must be contiguous on HW and
    # cannot read IO tensors directly)
    a2ax_in = nc.dram_tensor("a2ax_in", [NCORES, PV, RH, D], FH, kind="Internal")
    a2ax_out = nc.dram_tensor("a2ax_out", [NCORES, PV, RH, D], FH, kind="Internal")
    a2av_in = nc.dram_tensor("a2av_in", [NCORES, PV, RH, D], FH, kind="Internal")
    a2av_out = nc.dram_tensor("a2av_out", [NCORES, PV, RH, D], FH, kind="Internal")
    bn1_in = nc.dram_tensor("bn1_in", [C, 2], FP, kind="Internal")
    bn1_out = nc.dram_tensor("bn1_out", [C, 2], FP, kind="Internal", addr_space="Shared")
    bn2_in = nc.dram_tensor("bn2_in", [C, 2], FP, kind="Internal")
    bn2_out = nc.dram_tensor("bn2_out", [C, 2], FP, kind="Internal", addr_space="Shared")

    RG = [list(range(NCORES))]

    with tile.TileContext(nc) as tc, ExitStack() as top:
        pool_g = top.enter_context(tc.tile_pool(name="pg", bufs=1))   # small persistents
        psum_p = top.enter_context(tc.tile_pool(name="pp", bufs=1, space="PSUM"))

        # AllToAll #0: reshard x (W-shard -> H-shard+halo) from the
        # pre-windowed input parameter. HW collectives can't read IO
        # tensors, so one flat contiguous copy stages it. Emitted first
        # so CC starts immediately.
        nc.sync.dma_start(a2ax_in.ap(), xs.ap())
        nc.gpsimd.collective_compute(
            "AllToAll", ALU.bypass, replica_groups=RG,
            ins=[a2ax_in.ap().opt()], outs=[a2ax_out.ap().opt()],
        )

        def load_reshard(dst_tile, src_dram, rl0, nr, engines):
            """Load [(rl,c), w=(s,col), d] tile rows rl0..rl0+nr from the
            contiguous AllToAll output [s, (col,c), rl, d]. One DMA per
            row (DMA AP balancing caps at 3 dims), issued round-robin on
            `engines`' DMA queues."""
            src5 = src_dram.ap().rearrange("s (col c) rl d -> rl c s col d",
                                           c=C)
            for rl in range(nr):
                dst = dst_tile[C * rl:C * (rl + 1)] \
                    .rearrange("c (s col) d -> c s col d", col=CPC)
                engines[rl % len(engines)].dma_start(dst, src5[rl0 + rl])

        # ---------- helpers ----------
        def normalize_k(pool, kt, P, L, tag, kf_pool=None):
            """L1-normalize fp16 kt [P, L, 5] along the 5 axis into a new
            fp32 tile (scan scalar operands must be fp32)."""
            kf_pool = kf_pool or pool
            den = pool.tile([P, L], FP, name=f"den_{tag}", tag="den", bufs=3)
            nc.vector.tensor_reduce(
                out=den[:], in_=kt[:], axis=AX.X, op=ALU.add,
                apply_absolute_value=True,
            )
            nc.vector.tensor_scalar_max(den[:], den[:], EPS_L1)
            rec = pool.tile([P, L], FP, name=f"rec_{tag}", tag="rec", bufs=3)
            nc.vector.reciprocal(rec[:], den[:])
            kf = kf_pool.tile([P, L, 5], FP, name=f"kf_{tag}", tag=f"kf_{tag}")
            for j in range(5):
                nc.vector.tensor_mul(kf[:, :, j], kt[:, :, j], rec[:])
            return kf

        scan_counter = [0]

        def emit_scan(pool, x_g, kn_g, hist_g, L, reverse, P):
            """Sequential SGA scan along axis 'L' of x_g [P, L, D].

            hist_g [P, L, DP] fp16 holds the scan history with zero guard
            columns 0 and D+1 (pre-zeroed by caller). Per step: 3 fused
            custom DVE ops, all on VectorE.
            """
            cid = scan_counter[0]
            scan_counter[0] += 1
            mxA = pool.tile([P, 1], FP, name=f"mxA{cid}", tag=f"mxA{cid}")
            mxB = pool.tile([P, 1], FP, name=f"mxB{cid}", tag=f"mxB{cid}")
            at = pool.tile([P, D], FP, name=f"a{cid}", tag=f"a{cid}")
            bt = pool.tile([P, D], FP, name=f"b{cid}", tag=f"b{cid}")

            pos0 = L - 1 if reverse else 0
            nc.vector.tensor_copy(hist_g[:, pos0, 1:D + 1], x_g[:, pos0, :])
            nc.vector.tensor_reduce(out=mxA[:], in_=hist_g[:, pos0, 1:D + 1],
                                    axis=AX.X, op=ALU.max)

            mx_prev, mx_next = mxA, mxB
            for s in range(1, L):
                t = L - 1 - s if reverse else s
                tp = t + 1 if reverse else t - 1
                hp = hist_g[:, tp, :]          # padded prev row [P, DP]
                k0 = kn_g[:, t, 0:1]
                k1 = kn_g[:, t, 1:2]
                k2 = kn_g[:, t, 2:3]
                k3 = kn_g[:, t, 3:4]
                k4 = kn_g[:, t, 4:5]
                # a = x_t*w0 + prev*w2
                nc.vector._custom_dve(
                    DUAL, out=at[:], in0=x_g[:, t, :], in1=hp[:, 1:D + 1],
                    s0=k0, s1=k2)
                # b = prev[d-1]*w1 + prev[d+1]*w3 (guards make edges exact)
                nc.vector._custom_dve(
                    DUAL, out=bt[:], in0=hp[:, 0:D], in1=hp[:, 2:DP],
                    s0=k1, s1=k3)
                # cur = a + b + mx*w4 ; mx_next = max_d(cur)
                nc.vector._custom_dve(
                    TRI, out=hist_g[:, t, 1:D + 1], in0=at[:], in1=bt[:],
                    s0=mx_prev[:], s1=k4, accum_out=mx_next[:])
                mx_prev, mx_next = mx_next, mx_prev

        def zero_guards(hist_g, P, L):
            nc.vector.memset(hist_g[:, :, 0:1], 0.0)
            nc.vector.memset(hist_g[:, :, D + 1:DP], 0.0)

        # =======================================================
        # Phase V: vertical scans (W-sharded), feed AllToAll #1.
        # All loads are emitted up front: the SP queue is in-order, so
        # the phase-H loads must not sit behind the scan-gated result
        # writes; phase-H tiles load on the Scalar DMA queue instead.
        # =======================================================
        zrow = pool_g.tile([128, 1, D], FH, name="zrow", tag="zrow")
        nc.vector.memset(zrow[:], 0.0)
        pe = top.enter_context(tc.tile_pool(name="pe", bufs=1))
        HG = [(0, 128), (128, 128), (256, 64)]  # (row-part offset, partitions)
        xh_tiles, k12_tiles, kh_f = [], [], []
        with tc.tile_pool(name="pv", bufs=1) as pv:
            xv_tiles, k3_tiles, k4_tiles = [], [], []
            for g in range(4):
                # x rows for the vertical scan, gathered from the
                # windowed layout (row h lives in block h//8 at 1 + h%8)
                xv_g = pv.tile([128, H, D], FH, name=f"xv{g}", tag=f"xv{g}")
                src = AP(xs, (128 * g * RH + 1) * D,
                         [[RH * D, 128], [PV * RH * D, NCORES], [D, RPC],
                          [1, D]])
                nc.sync.dma_start(xv_g[:], src)
                xv_tiles.append(xv_g)
                k3_g = pv.tile([128, H, 5], FH, name=f"k3{g}", tag=f"k3{g}")
                nc.sync.dma_start(k3_g[:], k3v[128 * g:128 * (g + 1), :, :])
                k4_g = pv.tile([128, H, 5], FH, name=f"k4{g}", tag=f"k4{g}")
                nc.sync.dma_start(k4_g[:], k4v[128 * g:128 * (g + 1), :, :])
                k3_tiles.append(k3_g)
                k4_tiles.append(k4_g)
            # phase-H loads, emitted BEFORE any scan-gated DMA: the DMA
            # lanes (DMAHW0..7) are assigned round-robin in emission
            # order and each lane is in-order, so emitting these after
            # the chunk writes would chain them behind the V scans.
            for g, (p0, P) in enumerate(HG):
                xh_g = pe.tile([P, W, D], FH, name=f"xh{g}", tag=f"xh{g}")
                load_reshard(xh_g, a2ax_out, p0 // C, P // C,
                             engines=[nc.scalar])
                xh_tiles.append(xh_g)
                k1_g = pv.tile([P, W, 5], FH, name=f"k1{g}", tag=f"k1{g}")
                nc.scalar.dma_start(k1_g[:], k1h[p0:p0 + P, :, :])
                k2_g = pv.tile([P, W, 5], FH, name=f"k2{g}", tag=f"k2{g}")
                nc.scalar.dma_start(k2_g[:], k2h[p0:p0 + P, :, :])
                k12_tiles.append((k1_g, k2_g))
            for g in range(4):
                xv_g, k3_g, k4_g = xv_tiles[g], k3_tiles[g], k4_tiles[g]
                k3f = normalize_k(pv, k3_g, 128, H, f"v3{g}")
                k4f = normalize_k(pv, k4_g, 128, H, f"v4{g}")
                a3_g = pv.tile([128, H, DP], FH, name=f"a3{g}", tag=f"a3{g}")
                h4_g = pv.tile([128, H, DP], FH, name=f"h4{g}", tag=f"h4{g}")
                zero_guards(a3_g, 128, H)
                zero_guards(h4_g, 128, H)
                emit_scan(pv, xv_g, k3f, a3_g, H, reverse=False, P=128)
                emit_scan(pv, xv_g, k4f, h4_g, H, reverse=True, P=128)
                nc.vector.tensor_max(a3_g[:], a3_g[:], h4_g[:])
                # scatter result chunks into the AllToAll input (1-row
                # halo each side; zero out-of-range rows)
                for j in range(NCORES):
                    h0 = RPC * j - 1
                    dst = a2av_in[j, 128 * g:128 * (g + 1), :, :]
                    if h0 < 0:
                        nc.sync.dma_start(dst[:, 1:RH, :],
                                          a3_g[:, 0:RH - 1, 1:D + 1])
                        nc.sync.dma_start(dst[:, 0:1, :], zrow[:])
                    elif h0 + RH > H:
                        nc.sync.dma_start(dst[:, 0:RH - 1, :],
                                          a3_g[:, h0:H, 1:D + 1])
                        nc.sync.dma_start(dst[:, RH - 1:RH, :], zrow[:])
                    else:
                        nc.sync.dma_start(dst, a3_g[:, h0:h0 + RH, 1:D + 1])
            # normalize the horizontal k's here (after the V scans in DVE
            # queue order, just before the H scans need them); raw tiles
            # die with this pool, normalized fp32 tiles live in ph.
            for g, (p0, P) in enumerate(HG):
                k1_g, k2_g = k12_tiles[g]
                kh_f.append((normalize_k(pv, k1_g, P, W, f"h1{g}", kf_pool=pe),
                             normalize_k(pv, k2_g, P, W, f"h2{g}", kf_pool=pe)))

        nc.gpsimd.collective_compute(
            "AllToAll", ALU.bypass, replica_groups=RG,
            ins=[a2av_in.ap().opt()], outs=[a2av_out.ap().opt()],
        )

        # =======================================================
        # Phase H: horizontal scans (H-sharded, 10 rows w/ halo)
        # Overlaps AllToAll #1 (only depends on AllToAll #0).
        # =======================================================
        with ExitStack() as hs:
            ph = hs.enter_context(tc.tile_pool(name="ph", bufs=1))
            a1_tiles = []
            with tc.tile_pool(name="phx", bufs=1) as phx:
                for g, (p0, P) in enumerate(HG):
                    xh_g = xh_tiles[g]
                    k1f, k2f = kh_f[g]
                    a1_g = ph.tile([P, W, DP], FH, name=f"a1{g}", tag=f"a1{g}")
                    h2_g = phx.tile([P, W, DP], FH, name=f"h2{g}", tag=f"h2{g}")
                    zero_guards(a1_g, P, W)
                    zero_guards(h2_g, P, W)
                    emit_scan(phx, xh_g, k1f, a1_g, W, reverse=False, P=P)
                    emit_scan(phx, xh_g, k2f, h2_g, W, reverse=True, P=P)
                    nc.vector.tensor_max(a1_g[:], a1_g[:], h2_g[:])
                    a1_tiles.append(a1_g)

            # ===================================================
            # Combine + BN1 stats
            # ===================================================
            pcv = hs.enter_context(tc.tile_pool(name="pcv", bufs=1))
            bnp = psum_p.tile([C, 2], FP, name="bnp", tag="bnp")
            sel_tiles = []
            for nm, ap_, P in (("s0", selg0, 128), ("s1", selg1, 128), ("s2", selg2, 64)):
                selt = pool_g.tile([P, C], FP, name=f"sel{nm}", tag=f"sel{nm}")
                nc.sync.dma_start(selt[:], ap_[:])
                sel_tiles.append(selt)

            for g, (p0, P) in enumerate(HG):
                a1_g = a1_tiles[g]
                a34_g = ph.tile([P, W, D], FH, name=f"a34_{g}", tag="a34", bufs=2)
                load_reshard(a34_g, a2av_out, p0 // C, P // C,
                             engines=[nc.scalar, nc.sync])
                s12_g = pool_g.tile([P, 2], FP, name=f"bns{g}", tag=f"bns{g}")
                # ymax = max(a12, a34) in place, fused sum accumulate
                nc.vector._custom_dve(
                    MAXS, out=a1_g[:, :, 1:D + 1], in0=a1_g[:, :, 1:D + 1],
                    in1=a34_g[:], s0=0.0, s1=0.0, accum_out=s12_g[:, 0:1])
                # sum of squares (ScalarE; scratch output reuses a34_g)
                nc.scalar.activation(a34_g[:], a1_g[:, :, 1:D + 1], ACT.Square,
                                     accum_out=s12_g[:, 1:2])
                nc.tensor.matmul(bnp[:], lhsT=sel_tiles[g][:], rhs=s12_g[:],
                                 start=(g == 0), stop=(g == 2))

            bnst = pool_g.tile([C, 2], FP, name="bnst", tag="bnst")
            nc.vector.tensor_copy(bnst[:], bnp[:])
            nc.sync.dma_start(bn1_in[:], bnst[:])
            nc.gpsimd.collective_compute(
                "AllReduce", ALU.add, replica_groups=RG,
                ins=[bn1_in.ap().opt()], outs=[bn1_out.ap().opt()],
            )

            def bn_scale_bias(bn_out_dram, gam, bet, tag):
                """([C,1] scale, [C,1] bias) from AllReduce'd (sum, sumsq)."""
                st = pool_g.tile([C, 2], FP, name=f"bnr{tag}", tag=f"bnr{tag}")
                nc.sync.dma_start(st[:], bn_out_dram[:])
                gt = pool_g.tile([C, 1], FP, name=f"bng{tag}", tag=f"bng{tag}")
                nc.sync.dma_start(gt[:], gam[:])
                bt = pool_g.tile([C, 1], FP, name=f"bnb{tag}", tag=f"bnb{tag}")
                nc.sync.dma_start(bt[:], bet[:])
                mean = pool_g.tile([C, 1], FP, name=f"mean{tag}", tag=f"mean{tag}")
                nc.vector.tensor_scalar_mul(mean[:], st[:, 0:1], 1.0 / NTOT)
                var = pool_g.tile([C, 1], FP, name=f"var{tag}", tag=f"var{tag}")
                nc.vector.tensor_scalar_mul(var[:], st[:, 1:2], 1.0 / NTOT)
                msq = pool_g.tile([C, 1], FP, name=f"msq{tag}", tag=f"msq{tag}")
                nc.vector.tensor_mul(msq[:], mean[:], mean[:])
                nc.vector.tensor_tensor(out=var[:], in0=var[:], in1=msq[:],
                                        op=ALU.subtract)
                nc.vector.tensor_scalar_add(var[:], var[:], EPS_BN)
                sd = pool_g.tile([C, 1], FP, name=f"sd{tag}", tag=f"sd{tag}")
                nc.scalar.activation(sd[:], var[:], ACT.Sqrt)
                rs = pool_g.tile([C, 1], FP, name=f"rs{tag}", tag=f"rs{tag}")
                nc.vector.reciprocal(rs[:], sd[:])
                sc = pool_g.tile([C, 1], FP, name=f"sc{tag}", tag=f"sc{tag}")
                nc.vector.tensor_mul(sc[:], rs[:], gt[:])
                bi = pool_g.tile([C, 1], FP, name=f"bi{tag}", tag=f"bi{tag}")
                nc.vector.tensor_mul(bi[:], mean[:], sc[:])
                nc.vector.tensor_tensor(out=bi[:], in0=bt[:], in1=bi[:],
                                        op=ALU.subtract)
                return sc, bi

            sc1, bi1 = bn_scale_bias(bn1_out, bn1g, bn1b, "1")
            sc1b = pool_g.tile([128, 1], FP, name="sc1b", tag="sc1b")
            bi1b = pool_g.tile([128, 1], FP, name="bi1b", tag="bi1b")
            for b in range(4):
                nc.sync.dma_start(sc1b[C * b:C * (b + 1), :], sc1[:])
                nc.sync.dma_start(bi1b[C * b:C * (b + 1), :], bi1[:])
            # y = relu(scale*ymax + bias), in place (halo rows included)
            for g, (p0, P) in enumerate(HG):
                nc.vector._custom_dve(
                    AFFR, out=a1_tiles[g][:, :, 1:D + 1],
                    in0=a1_tiles[g][:, :, 1:D + 1],
                    s0=sc1b[0:P, :], s1=bi1b[0:P, :])

            # ===================================================
            # Conv 3x3x3, row-folded: K=128 (4ry x 32ci), M=64
            # (2rsh x 32co), 9 (dz,dx) taps, D in 3 chunks of 16.
            # Result lands in SBUF tiles conv_sbuf[g] [(4row,32ch),W,D].
            # ===================================================
            wst = pool_g.tile([128, 9, 64], FH, name="wst", tag="wst")
            nc.sync.dma_start(wst[:], wstk[:])
            # conv result tiles, indexed like the xh halo window (block =
            # xh local row % 4, tile = row // 4; out row R <-> xh row R+1)
            # so the residual add and the BN1 sel masks line up directly.
            conv_sbuf = [
                ph.tile([P, W, D], FH, name=f"cs{g}", tag=f"cs{g}")
                for g, (_, P) in enumerate(HG)
            ]
            nc.vector.memset(conv_sbuf[0][0:C], 0.0)    # unused halo blocks
            nc.vector.memset(conv_sbuf[2][C:2 * C], 0.0)

            DC = 16
            DX = DC + 2                      # depth window incl dz shifts
            WP = W + 2
            taps = [(a, b) for a in range(3) for b in range(3)]
            for ch in range(3):
                d0 = DC * ch
                # y5[(ry,ci), rb, wp, dpx] = ypad[ci, d0+dpx-1, rb*2+ry, wp-1]
                # (dpx alloc 19, 18 used, to block wp/dpx AP coalescing; a1
                # zero-guard cols provide the depth edge padding)
                y5 = pcv.tile([128, 4, WP, DX + 1], FH, name=f"y5_{ch}",
                              tag="y5", bufs=2)
                nc.vector.memset(y5[:, :, 0:1, :], 0.0)
                nc.vector.memset(y5[:, :, WP - 1:WP, :], 0.0)
                for rb in range(4):
                    rbase = 1 + 2 * rb       # local a1 row of first output
                    for ry in range(4):
                        rr = rbase + ry - 1  # source local row (0..9)
                        g = rr // 4
                        rl = rr - 4 * g
                        src = a1_tiles[g][C * rl:C * (rl + 1), :, d0:d0 + DX]
                        dst = y5[32 * ry:32 * (ry + 1), rb, 1:W + 1, 0:DX]
                        eng = nc.sync if (rb % 2 == 0) else nc.scalar
                        eng.dma_start(dst, src)
                for rb in range(4):
                    for dps in range(4):     # 4 depths per matmul group
                        pt = psum_p.tile([64, W, 4], FP, name="cps", tag="cps",
                                         bufs=6)
                        for ti, (dz, dx) in enumerate(taps):
                            rhs = y5[:, rb, dx:dx + W,
                                     4 * dps + dz:4 * dps + dz + 4]
                            nc.tensor.matmul(pt[:], lhsT=wst[:, ti, :],
                                             rhs=rhs, start=(ti == 0),
                                             stop=(ti == 8))
                        dlo = d0 + 4 * dps
                        for rsh in range(2):
                            rr = 2 * rb + rsh + 1      # xh-window row (1..8)
                            go, prow = rr // 4, (rr % 4) * C
                            # PSUM drain on VectorE: idle during conv,
                            # and 3x cheaper than ScalarE per copy
                            # (GPSIMD cannot access PSUM on HW).
                            nc.vector.tensor_copy(
                                conv_sbuf[go][prow:prow + C, :, dlo:dlo + 4],
                                pt[32 * rsh:32 * (rsh + 1), :, :])

            # ===================================================
            # BN2 stats + final residual (layout [(4row,32ch),W,D])
            # ===================================================
            bnp2 = psum_p.tile([C, 2], FP, name="bnp2", tag="bnp2")
            for g, (p0, P) in enumerate(HG):
                sAB = pool_g.tile([P, 2], FP, name=f"sAB{g}", tag=f"sAB{g}")
                nc.vector.tensor_reduce(out=sAB[:, 0:1], in_=conv_sbuf[g][:],
                                        axis=AX.XY, op=ALU.add)
                sqs = ph.tile([P, W, D], FH, name=f"sq{g}", tag="a34",
                              bufs=2)
                nc.scalar.activation(sqs[:], conv_sbuf[g][:], ACT.Square,
                                     accum_out=sAB[:, 1:2])
                nc.tensor.matmul(bnp2[:], lhsT=sel_tiles[g][:], rhs=sAB[:],
                                 start=(g == 0), stop=(g == 2))
            bnst2 = pool_g.tile([C, 2], FP, name="bnst2", tag="bnst2")
            nc.vector.tensor_copy(bnst2[:], bnp2[:])
            nc.sync.dma_start(bn2_in[:], bnst2[:])
            nc.gpsimd.collective_compute(
                "AllReduce", ALU.add, replica_groups=RG,
                ins=[bn2_in.ap().opt()], outs=[bn2_out.ap().opt()],
            )
            sc2, bi2 = bn_scale_bias(bn2_out, bn2g, bn2b, "2")
            sc2b = pool_g.tile([128, 1], FP, name="sc2b", tag="sc2b")
            bi2b = pool_g.tile([128, 1], FP, name="bi2b", tag="bi2b")
            for b in range(4):
                nc.sync.dma_start(sc2b[C * b:C * (b + 1), :], sc2[:])
                nc.sync.dma_start(bi2b[C * b:C * (b + 1), :], bi2[:])

            # out = relu(sc2*conv + rem + bi2); conv tiles are xh-aligned,
            # so the residual is a direct per-partition add. Partition
            # quadrant rule: ops may start only at 0/32/64/96 with <=
            # 128/32/64/32 partitions, so cs0's valid span 32..128 splits.
            def stt(cs, xh_t, lo, hi):
                nc.vector.scalar_tensor_tensor(
                    out=cs[lo:hi], in0=cs[lo:hi], scalar=sc2b[lo:hi],
                    in1=xh_t[lo:hi], op0=ALU.mult, op1=ALU.add,
                )
            stt(conv_sbuf[0], xh_tiles[0], C, 2 * C)
            stt(conv_sbuf[0], xh_tiles[0], 2 * C, 4 * C)
            stt(conv_sbuf[1], xh_tiles[1], 0, 4 * C)
            stt(conv_sbuf[2], xh_tiles[2], 0, C)
            for g, (p0, P) in enumerate(HG):
                nc.scalar.activation(conv_sbuf[g][:], conv_sbuf[g][:],
                                     ACT.Relu, bias=bi2b[0:P], scale=1.0)
            # ship owned rows only (xh rows 1..8 = out rows 0..7),
            # spread across the three DMA-capable engine queues
            nc.sync.dma_start(out_d[0:96], conv_sbuf[0][C:4 * C])
            nc.scalar.dma_start(out_d[96:224], conv_sbuf[1][:])
            nc.gpsimd.dma_start(out_d[224:256], conv_sbuf[2][0:C])

    return nc


def _split_sync_waits(nc, maxw=1):
    """Walrus codegen rejects >1 sem wait per instruction (TPB_CTRL limit);
    hoist extras onto fresh NoOps on the same engine just before the owner."""
    from concourse import mybir
    n_split = 0
    for bb in nc.main_func.blocks:
        i = 0
        insts = bb.instructions
        while i < len(insts):
            ins = insts[i]
            si = getattr(ins, "sync_info", None)
            if si is not None and len(si.on_wait) > maxw:
                waits = list(si.on_wait)
                si.on_wait = waits[:maxw]
                extra = waits[maxw:]
                new_nops = []
                for j in range(0, len(extra), maxw):
                    nop = mybir.InstNoOp(
                        name=nc.get_next_instruction_name(),
                        sync_info=mybir.SyncInfo(
                            on_wait=extra[j:j + maxw], on_update=[]),
                        bass_nofuse=True,
                        engine=ins.engine,
                    )
                    nc.register_instruction(nop)
                    new_nops.append(nop)
                for k, nop in enumerate(new_nops):
                    insts.insert(i + k, nop)
                i += len(new_nops)
                n_split += 1
            i += 1
    return n_split


def _get_program():
    if "nc" not in _prog_cache:
        from concourse import mybir
        nc = _build_program()
        _split_sync_waits(nc)
        # Raw Bass doesn't encode InstISA subclasses (custom DVE etc.);
        # walrus then sees empty .instr -> "ISA wrong length".
        mybir.codegen_inst_isa_subclasses(nc)
        _prog_cache["nc"] = nc
    return _prog_cache["nc"]


def _prep_inputs(x, g, conv_w, bn1_gamma, bn1_beta, bn2_gamma, bn2_beta):
    """Host-side sharding + layout transposes + fp16 cast (no arithmetic)."""
    x = np.asarray(x, np.float32)[0]            # [C,D,H,W]
    g = np.asarray(g, np.float32)[0]            # [640,H,W]
    conv_w = np.asarray(conv_w, np.float32)
    ks = g.reshape(4, C, 5, H, W)               # k1..k4

    # conv weight stack, row-folded:
    # wstk[ry*32+ci, dz*3+dx, rsh*32+co] = conv_w[co,ci,dz,ry-rsh,dx]
    wstk = np.zeros((128, 9, 64), np.float32)
    for ry in range(4):
        for rsh in range(2):
            dy = ry - rsh
            if 0 <= dy <= 2:
                # [ci, (dz,dx), co]
                wblk = conv_w[:, :, :, dy, :].transpose(1, 2, 3, 0) \
                    .reshape(C, 9, C)
                wstk[32 * ry:32 * (ry + 1), :, 32 * rsh:32 * (rsh + 1)] = wblk
    wstk = wstk.astype(np.float16)

    # BN1 masks per h-group (owned local rows are 1..8 of 0..9)
    def sel(nrows, r_base):
        m = np.zeros((nrows * C, C), np.float32)
        for rl in range(nrows):
            r = r_base + rl
            if 1 <= r <= 8:
                for c in range(C):
                    m[rl * C + c, c] = 1.0
        return m
    selg0, selg1, selg2 = sel(4, 0), sel(4, 4), sel(2, 8)

    maps = []
    for i in range(NCORES):
        r_lo, r_hi = 8 * i - 1, 8 * i + 9
        k1p = np.zeros((RH, C, W, 5), np.float16)
        k2p = np.zeros((RH, C, W, 5), np.float16)
        lo, hi = max(r_lo, 0), min(r_hi, H)
        k1p[lo - r_lo:hi - r_lo] = ks[0][:, :, lo:hi, :].transpose(2, 0, 3, 1)
        k2p[lo - r_lo:hi - r_lo] = ks[1][:, :, lo:hi, :].transpose(2, 0, 3, 1)
        cs = slice(CPC * i, CPC * (i + 1))
        xpad = np.zeros((PV, H + 2, D), np.float16)
        xpad[:, 1:H + 1, :] = x[:, :, :, cs].transpose(3, 0, 2, 1) \
            .reshape(PV, H, D)
        widx = RPC * np.arange(NCORES)[:, None] + np.arange(RH)[None, :]
        xsw = np.ascontiguousarray(xpad[:, widx, :].transpose(1, 0, 2, 3))
        k3s = ks[2][:, :, :, cs].transpose(3, 0, 2, 1).reshape(PV, H, 5)
        k4s = ks[3][:, :, :, cs].transpose(3, 0, 2, 1).reshape(PV, H, 5)
        maps.append({
            "xs": xsw,
            "k1h": np.ascontiguousarray(k1p.reshape(PH, W, 5)),
            "k2h": np.ascontiguousarray(k2p.reshape(PH, W, 5)),
            "k3v": np.ascontiguousarray(k3s.astype(np.float16)),
            "k4v": np.ascontiguousarray(k4s.astype(np.float16)),
            "wstk": wstk,
            "bn1g": np.asarray(bn1_gamma, np.float32).reshape(C, 1),
            "bn1b": np.asarray(bn1_beta, np.float32).reshape(C, 1),
            "bn2g": np.asarray(bn2_gamma, np.float32).reshape(C, 1),
            "bn2b": np.asarray(bn2_beta, np.float32).reshape(C, 1),
            "selg0": selg0, "selg1": selg1, "selg2": selg2,
        })
    return maps


def _assemble(results):
    """Per-core out [256, W, D] fp16 ((row,ch) layout) -> full [1,C,D,H,W]."""
    full = np.zeros((1, C, D, H, W), np.float32)
    for i, r in enumerate(results):
        o = np.asarray(r["out"]).astype(np.float32) \
             .reshape(RPC, C, W, D).transpose(1, 3, 0, 2)
        full[0, :, :, 8 * i:8 * (i + 1), :] = o
    return full


def _run_sim(nc, maps):
    """Fallback: numerically-exact multi-core simulator (no hardware)."""
    from concourse import bass_interp
    sim = bass_interp.MultiCoreSim(nc, NCORES)
    for i in range(NCORES):
        for name, arr in maps[i].items():
            sim.cores[i].tensor(name)[:] = arr
    sim.simulate(check_with_hw=False)
    return [{"out": sim.cores[i].mem_tensor("out")} for i in range(NCORES)]


def kernel(x, g, conv_w, bn1_gamma, bn1_beta, bn2_gamma, bn2_beta):
    from concourse.bass_utils import run_bass_kernel_spmd
    nc = _get_program()
    maps = _prep_inputs(x, g, conv_w, bn1_gamma, bn1_beta, bn2_gamma, bn2_beta)
    try:
        res = run_bass_kernel_spmd(nc, maps, list(range(NCORES)))
        results = res.results
    except Exception as e:  # axon/PJRT unavailable -> simulate
        print(f"kernel: hardware path failed ({type(e).__name__}: {e}); "
              f"falling back to MultiCoreSim", file=sys.stderr)
        results = _run_sim(nc, maps)
    return _assemble(results)


if __name__ == "__main__":
    nc = _build_program()
    print("program built OK; instructions:",
          sum(len(bb.instructions) for bb in nc.main_func.blocks))
